# revision 68
# baseline (speedup 1.0000x reference)
"""DeepONet (branch MLP + LoRA-generated per-sample trunk) on 8 TRN2 cores.

Data-parallel over batch: each of the 8 NeuronCores processes 256 samples
(two 128-sample tiles).  All matmul operands are bf16 (fp32 PSUM accumulate);
measured end-to-end absmax error vs the fp32 reference is ~5e-4 on an
output scale of ~3.3.  Branch layer 4 is linear and only feeds the LoRA
projection, so V = bw4 @ W1 and W1^T bb4 are folded on the host (exact).

Per-sample trunk mid-layers use the LoRA form W_l[b] = sum_k c[b,k] * A_k
with shared A_k = reshape(W2 rows).  The per-sample k-contraction
h2[o,b] = sum_k Y_k[b,o] * c[b,k]  (Y_k = h1 @ A_k) runs entirely on the PE
as a PSUM-accumulated chain of diagonal matmuls: lhsT = Y_k slice, rhs =
diag(c[:,k]) (built once per batch-tile on DVE/GPSIMD from an identity).
The output lands FEATURE-major, so it feeds the next layer's Y matmuls
(lhsT = hF) with no transpose anywhere in the trunk.  Each layer's bias is
folded in as the accumulation group's first matmul (lhsT = W2 bias columns,
rhs = cF tile), and trunk layer 0's t-scaling is itself a diag(t) matmul.
The final per-sample dot product is a ones-vector matmul partition-reduce.
"""

import numpy as np
import ml_dtypes

BF = ml_dtypes.bfloat16

N_CORES = 8
B = 2048
BL = B // N_CORES          # 256 samples per core
SENSOR = 128
UNITS = 1024
LORA = 64
TU = 128

# trunk param offsets within P=33409
L1B_OFF = 256
L1W_OFF = 384
L2B_OFF = 16768
L2W_OFF = 16896
L3_OFF = 33280


# ---------------------------------------------------------------------------
# Walrus here accepts only ONE sync-wait command per instruction; Tile's wait
# assigner attaches several.  Split extras onto standalone EVSEM waits.
# ---------------------------------------------------------------------------
def _install_waitfix():
    import bass_rust as _bass_rust
    import concourse.tile as _tile
    import concourse.mybir as mybir
    from concourse.vector_clock import ScopedClock

    if getattr(_tile.TileContext, "_waitfix_installed", False):
        return

    _MODES = {"sem-ge-imm": "sem-ge", "sem-ge": "sem-ge"}

    def _split(tc, inst):
        si = inst.sync_info
        if si is None or not si.on_wait or len(si.on_wait) <= 1:
            return
        waits = list(si.on_wait)
        keep_idx = 0
        for i, w in enumerate(waits):
            if w.wait_mode not in _MODES or w.wait_reg is not None:
                keep_idx = i
                break
        keep = waits.pop(keep_idx)
        for w in waits:
            assert w.wait_mode in _MODES and w.wait_reg is None
        si.on_wait = [keep]
        inst.sync_info = si
        eng = tc.nc.engines[inst.engine]
        for w in waits:
            sem = _bass_rust.SemaphoreHandle(name=w.ant_name, num=w.id)
            eng.wait_op(sem, int(w.wait_value), _MODES[w.wait_mode])

    _orig_commit = _tile.TileContext._commit_instruction
    _orig_exit_unused = _tile.TileContext._drain_and_barrier

    def _patched_commit(self, inst, lazy_reg_writes=True):
        si = inst.sync_info
        if (
            si is not None
            and si.on_wait
            and len(si.on_wait) > 1
            and inst.engine != mybir.EngineType.Unassigned
        ):
            cb = self.nc._state.pop_inst_callback()
            try:
                _split(self, inst)
            finally:
                self.nc._state.push_inst_callback(cb)
        return _orig_commit(self, inst, lazy_reg_writes=lazy_reg_writes)

    def _patched_drain(self, tick_clock, wait_clock):
        drain_inst = self.nc.sync.drain()
        wait_clock.add_sem_waits(
            drain_inst.ins, ScopedClock({None: tick_clock.global_clock})
        )
        _split(self, drain_inst.ins)
        self.nc.all_engine_barrier()
        assert self.sems is not None
        popped = self.nc._tile_sem_poison_stack.pop()
        assert popped is self._sem_poison
        self.nc.clear_and_free_semaphores(list(self.sems.allocated().values()))
        self.nc.all_engine_barrier()

    _tile.TileContext._commit_instruction = _patched_commit
    _tile.TileContext._drain_and_barrier = _patched_drain
    _tile.TileContext._waitfix_installed = True


# ---------------------------------------------------------------------------
# Bass program (built once, cached)
# ---------------------------------------------------------------------------
_PROGRAM = None


def _build_program():
    _install_waitfix()
    from contextlib import ExitStack

    import concourse.bass as bass
    import concourse.mybir as mybir
    from concourse.tile import TileContext

    dt = mybir.dt
    AF = mybir.ActivationFunctionType
    OP = mybir.AluOpType

    nc = bass.Bass(
        trn_type="TRN2", target_bir_lowering=False, debug=False,
        num_devices=N_CORES,
    )

    # ---- DRAM I/O ----
    uF_d = nc.dram_tensor("uF", [128, BL], dt.bfloat16, kind="ExternalInput")
    u0_d = nc.dram_tensor("u0", [128, 2], dt.float32, kind="ExternalInput")
    tb_d = nc.dram_tensor("tb", [128, 2], dt.float32, kind="ExternalInput")
    bw0_d = nc.dram_tensor("bw0", [128, UNITS], dt.bfloat16, kind="ExternalInput")
    bw_d = [
        nc.dram_tensor(f"bw{i}", [UNITS, UNITS], dt.bfloat16, kind="ExternalInput")
        for i in range(1, 4)
    ]
    bb_d = nc.dram_tensor("bb", [128, 40], dt.float32, kind="ExternalInput")
    W1_d = nc.dram_tensor("W1", [128, 8 * LORA], dt.bfloat16, kind="ExternalInput")
    cb4_d = nc.dram_tensor("cb4", [1, LORA], dt.bfloat16, kind="ExternalInput")
    A_d = [
        nc.dram_tensor(nm, [128, LORA * 128], dt.bfloat16, kind="ExternalInput")
        for nm in ("A1", "A2")
    ]
    w2l0_d = nc.dram_tensor("w2l0", [LORA, 384], dt.bfloat16, kind="ExternalInput")
    w2l2b_d = nc.dram_tensor("w2l2b", [LORA, 128], dt.bfloat16, kind="ExternalInput")
    w2l3_d = nc.dram_tensor("w2l3", [LORA, 129], dt.bfloat16, kind="ExternalInput")
    out_d = nc.dram_tensor("out", [128, 2], dt.float32, kind="ExternalOutput")

    with TileContext(nc) as tc, ExitStack() as ctx:
        # ---- SBUF pools ----
        wpool = ctx.enter_context(tc.tile_pool(name="weights", bufs=1))
        apool = ctx.enter_context(tc.tile_pool(name="acts", bufs=2))
        spool = ctx.enter_context(tc.tile_pool(name="small", bufs=1))
        ypool = ctx.enter_context(tc.tile_pool(name="ysb", bufs=3))
        hpool = ctx.enter_context(tc.tile_pool(name="hsb", bufs=4))

        # ---- weight loads ----
        # Small tensors first: branch L0 needs uF/bw0/bb immediately and a
        # multi-MB DMA queued ahead of them would stall the whole pipeline.
        uF = wpool.tile([128, BL], dt.bfloat16, name="uF_sb")
        nc.sync.dma_start(out=uF[:, :], in_=uF_d[:, :])
        bw0 = wpool.tile([128, UNITS], dt.bfloat16, name="bw0_sb")
        nc.sync.dma_start(out=bw0[:, :], in_=bw0_d[:, :])
        bb = spool.tile([128, 40], dt.float32, name="bb_sb")
        nc.sync.dma_start(out=bb[:, :], in_=bb_d[:, :])
        W1 = spool.tile([128, 8, LORA], dt.bfloat16, name="W1_sb")
        nc.gpsimd.dma_start(
            out=W1[:, :, :], in_=W1_d.rearrange("p (k m) -> p k m", k=8)
        )
        cb4_sb = spool.tile([1, LORA], dt.bfloat16, name="cb4_sb")
        nc.gpsimd.dma_start(out=cb4_sb[:, :], in_=cb4_d[:, :])
        # Big weights, in consumption order so each layer's weights land
        # just ahead of its matmuls.  Trunk-only small tensors are issued
        # between bw2 and bw3 — early enough for the trunk (~40us), late
        # enough not to delay bw1/bw2 on the shared DMA engines.
        bws = []
        for i in range(3):
            t = wpool.tile([128, 8, UNITS], dt.bfloat16, name=f"bw{i+1}_sb")
            src = bw_d[i].rearrange("(k p) m -> p k m", p=128)
            for k in range(8):
                nc.sync.dma_start(out=t[:, k, :], in_=src[:, k, :])
            bws.append(t)
            if i == 1:
                w2l0 = spool.tile([LORA, 384], dt.bfloat16, name="w2l0_sb")
                nc.gpsimd.dma_start(out=w2l0[:, :], in_=w2l0_d[:, :])
                w2l2b = spool.tile([LORA, 128], dt.bfloat16, name="w2l2b_sb")
                nc.gpsimd.dma_start(out=w2l2b[:, :], in_=w2l2b_d[:, :])
                w2l3 = spool.tile([LORA, 129], dt.bfloat16, name="w2l3_sb")
                nc.gpsimd.dma_start(out=w2l3[:, :], in_=w2l3_d[:, :])
                u0 = spool.tile([128, 2], dt.float32, name="u0_sb")
                nc.gpsimd.dma_start(out=u0[:, :], in_=u0_d[:, :])
                tb = spool.tile([128, 2], dt.float32, name="tb_sb")
                nc.gpsimd.dma_start(out=tb[:, :], in_=tb_d[:, :])
        A_sb = []
        for i in range(2):
            t = wpool.tile([128, LORA * 128], dt.bfloat16, name=f"A{i+1}_sb")
            nc.sync.dma_start(out=t[:, :], in_=A_d[i][:, :])
            A_sb.append(t)

        # identity (bf16) for transposes and diag building
        iota_i = spool.tile([128, 128], dt.int32, name="iota_sb")
        nc.gpsimd.iota(iota_i[:, :], [[1, 128]], base=0, channel_multiplier=-1)
        I_bf = spool.tile([128, 128], dt.bfloat16, name="ident_sb")
        nc.vector.tensor_scalar(I_bf[:, :], iota_i[:, :], 0, None, OP.is_equal)
        ones_bf = spool.tile([128, 1], dt.bfloat16, name="ones_sb")
        nc.vector.memset(ones_bf[:, :], 1.0)
        ones_row = spool.tile([1, BL], dt.bfloat16, name="onesr_sb")
        nc.vector.memset(ones_row[:, :], 1.0)

        # ---- branch MLP (feature-major) ----
        with tc.tile_pool(name="bpsum", bufs=4, space="PSUM") as bpsum:

            def branch_layer(prev_tiles, w_sb, l, final=False):
                """prev_tiles: list of k-tile APs [128, BL]; returns act tile."""
                nxt = apool.tile([128, 8, BL], dt.bfloat16, name=f"act{l}",
                                 tag="acts")
                for m in range(8):
                    ps = bpsum.tile([128, BL], dt.float32, name=f"ps{l}_{m}",
                                    tag="bps")
                    for k, pk in enumerate(prev_tiles):
                        lhsT = (w_sb[:, m * 128:(m + 1) * 128] if l == 0
                                else w_sb[:, k, m * 128:(m + 1) * 128])
                        nc.tensor.matmul(
                            ps[:, :], lhsT, pk,
                            start=(k == 0), stop=(k == len(prev_tiles) - 1),
                        )
                    nc.scalar.activation(
                        nxt[:, m, :], ps[:, :],
                        AF.Identity if final else AF.Tanh,
                        bias=bb[:, l * 8 + m:l * 8 + m + 1],
                        scale=1.0,
                    )
                return nxt

            act = branch_layer([uF[:, :]], bw0, 0)
            for l in range(1, 4):
                prev = [act[:, k, :] for k in range(8)]
                act = branch_layer(prev, bws[l - 1], l)
            net = act  # [128, 8, BL]  (act3: branch L4 is linear and only
            # feeds c, so V = bw4 @ W1 and cb4 = W1^T bb4 are folded on the
            # host; c = act3 @ V + cb4 exactly)

            # ---- c in both layouts (bias via K=1 ones-matmul) ----
            cF = spool.tile([LORA, BL], dt.bfloat16, name="cF_sb")
            ps_cF = bpsum.tile([LORA, BL], dt.float32, name="ps_cF", tag="bps")
            nc.tensor.matmul(ps_cF[:, :], cb4_sb[:, :], ones_row[:, :],
                             start=True, stop=False)
            for k in range(8):
                nc.tensor.matmul(ps_cF[:, :], W1[:, k, :], net[:, k, :],
                                 start=False, stop=(k == 7))
            nc.vector.tensor_copy(cF[:, :], ps_cF[:, :])

            c_bm = spool.tile([128, 2, LORA], dt.float32, name="cbm_sb")
            for j in range(2):
                ps_c = bpsum.tile([128, LORA], dt.float32, name=f"ps_c{j}",
                                  tag="bps")
                nc.tensor.matmul(ps_c[:, :], ones_row[:, 0:128],
                                 cb4_sb[:, :], start=True, stop=False)
                for k in range(8):
                    nc.tensor.matmul(
                        ps_c[:, :], net[:, k, j * 128:(j + 1) * 128],
                        W1[:, k, :], start=False, stop=(k == 7),
                    )
                nc.vector.tensor_copy(c_bm[:, j, :], ps_c[:, :])

        # ---- trunk ----
        D_sb = wpool.tile([128, 2, LORA * 128], dt.bfloat16, name="D_sb")
        out_sb = spool.tile([128, 2], dt.float32, name="out_sb")

        with (
            tc.tile_pool(name="ypsum", bufs=3, space="PSUM") as ypsum,
            tc.tile_pool(name="hpsum", bufs=2, space="PSUM") as hpsum,
            tc.tile_pool(name="mpsum", bufs=2, space="PSUM") as mpsum,
        ):
            # All trunk activations are kept FEATURE-major [feat, batch].
            # Mid layers: ps_hF[o, b] = sum_k Y_k[b, o] * c[b, k] via
            # matmul(lhsT=y_sb slice [b', o], rhs=D_k [b', b]), so the layer
            # output needs no transpose before feeding the next layer's
            # Y matmuls (lhsT = hF).
            for j in range(2):
                cFt = cF[:, j * 128:(j + 1) * 128]
                t_col = tb[:, j:j + 1]

                # trunk layer 0 (feature-major):
                #   hF1[o,b] = tanh(t[b]*W0[b,o] + B0[o,b])
                ps_l0 = mpsum.tile([128, 128], dt.float32, name=f"psl0_{j}",
                                   tag="mps")
                nc.tensor.matmul(ps_l0[:, :], cFt, w2l0[:, 128:256])
                w0sb = hpool.tile([128, 128], dt.bfloat16, name=f"w0sb_{j}",
                                  tag="w0sb")
                nc.scalar.copy(w0sb[:, :], ps_l0[:, :])
                Dt = hpool.tile([128, 128], dt.bfloat16, name=f"Dt_{j}",
                                tag="Dt")
                nc.vector.tensor_scalar(Dt[:, :], I_bf[:, :], t_col, None,
                                        OP.mult)
                ps_h1 = hpsum.tile([128, 128], dt.float32, name=f"psh1_{j}",
                                   tag="hps")
                nc.tensor.matmul(ps_h1[:, :], w2l0[:, 0:128], cFt,
                                 start=True, stop=False)
                nc.tensor.matmul(ps_h1[:, :], w0sb[:, :], Dt[:, :],
                                 start=False, stop=True)
                hF = hpool.tile([128, 128], dt.bfloat16, name=f"h1F_{j}",
                                tag="hF")
                nc.scalar.activation(hF[:, :], ps_h1[:, :], AF.Tanh)

                # diag matrices for this batch tile; emitted after the l0
                # chain so the scheduler prioritizes the l0 ops.
                for k in range(LORA):
                    eng = nc.vector if k % 2 == 0 else nc.gpsimd
                    eng.tensor_scalar(
                        D_sb[:, j, k * 128:(k + 1) * 128], I_bf[:, :],
                        c_bm[:, j, k:k + 1], None, OP.mult,
                    )

                # trunk mid layers (output feature-major directly)
                for l in range(2):
                    ps_h2 = hpsum.tile([128, 128], dt.float32,
                                       name=f"psh2_{l}_{j}", tag="hps")
                    # bias as first matmul of the accumulation group:
                    # bias_F[o,b] = sum_k W2[k, boff+o] * cF[k, b]
                    blhs = w2l0[:, 256:384] if l == 0 else w2l2b[:, :]
                    nc.tensor.matmul(ps_h2[:, :], blhs, cFt,
                                     start=True, stop=False)
                    for chunk in range(16):
                        ps_y = ypsum.tile([128, 512], dt.float32,
                                          name=f"psy{l}_{j}_{chunk}", tag="yps")
                        nc.tensor.matmul(
                            ps_y[:, :], hF[:, :],
                            A_sb[l][:, chunk * 512:(chunk + 1) * 512],
                        )
                        y_sb = ypool.tile([128, 512], dt.bfloat16,
                                          name=f"ysb{l}_{j}_{chunk}", tag="ysb")
                        if chunk % 4 != 1:
                            nc.scalar.copy(y_sb[:, :], ps_y[:, :])
                        else:
                            nc.vector.tensor_copy(y_sb[:, :], ps_y[:, :])
                        for kk in range(4):
                            k = chunk * 4 + kk
                            nc.tensor.matmul(
                                ps_h2[:, :],
                                y_sb[:, kk * 128:(kk + 1) * 128],
                                D_sb[:, j, k * 128:(k + 1) * 128],
                                start=False, stop=(k == LORA - 1),
                            )
                    hF = hpool.tile([128, 128], dt.bfloat16,
                                    name=f"h{l+2}F_{j}", tag="hF")
                    nc.scalar.activation(hF[:, :], ps_h2[:, :], AF.Tanh)

                # trunk layer 3 + output (q via ones-matmul partition reduce)
                ps_w3 = mpsum.tile([128, 128], dt.float32, name=f"psw3_{j}",
                                   tag="mps")
                nc.tensor.matmul(ps_w3[:, :], w2l3[:, 1:129], cFt)
                prod = hpool.tile([128, 128], dt.bfloat16, name=f"prod_{j}",
                                  tag="prod")
                nc.vector.tensor_tensor(prod[:, :], ps_w3[:, :], hF[:, :],
                                        OP.mult)
                ps_q = mpsum.tile([128, 1], dt.float32, name=f"psq_{j}",
                                  tag="qps", bufs=1)
                nc.tensor.matmul(ps_q[:, :], cFt, w2l3[:, 0:1],
                                 start=True, stop=False)
                nc.tensor.matmul(ps_q[:, :], prod[:, :], ones_bf[:, :],
                                 start=False, stop=True)
                nc.vector.scalar_tensor_tensor(
                    out_sb[:, j:j + 1], ps_q[:, :], t_col, u0[:, j:j + 1],
                    OP.mult, OP.add,
                )
            nc.sync.dma_start(out=out_d[:, :], in_=out_sb[:, :])

    return nc


def _get_program():
    global _PROGRAM
    if _PROGRAM is None:
        _PROGRAM = _build_program()
    return _PROGRAM


# ---------------------------------------------------------------------------
# host-side prep / gather
# ---------------------------------------------------------------------------
def _host_prep(inputs, core):
    s = slice(core * BL, (core + 1) * BL)
    u = np.asarray(inputs["u"][s], np.float32)
    t = np.asarray(inputs["t"][s], np.float32)
    W2 = np.asarray(inputs["W2"], np.float32)
    bf = lambda x: np.ascontiguousarray(np.asarray(x, np.float32)).astype(BF)
    d = {
        "uF": bf(u.T),
        "u0": np.ascontiguousarray(u[:, 0].reshape(2, 128).T),
        "tb": np.ascontiguousarray(t.reshape(2, 128).T),
        "bw0": bf(inputs["bw0"]),
        "W1": None,  # filled below (V = bw4 @ W1, prearranged)
        "w2l0": bf(W2[:, 0:384]),
        "w2l2b": bf(W2[:, L2B_OFF:L2B_OFF + 128]),
        "w2l3": bf(W2[:, L3_OFF:L3_OFF + 129]),
    }
    for i in range(1, 4):
        d[f"bw{i}"] = bf(inputs[f"bw{i}"])
    W1f = np.asarray(inputs["W1"], np.float64)
    V = (np.asarray(inputs["bw4"], np.float64) @ W1f).astype(np.float32)
    d["W1"] = bf(V.reshape(8, 128, LORA).transpose(1, 0, 2)
                 .reshape(128, 8 * LORA))
    d["cb4"] = bf((W1f.T @ np.asarray(inputs["bb4"], np.float64))
                  .astype(np.float32).reshape(1, LORA))
    bb = np.zeros((128, 40), np.float32)
    for l in range(5):
        bb[:, l * 8:(l + 1) * 8] = np.asarray(
            inputs[f"bb{l}"], np.float32).reshape(8, 128).T
    d["bb"] = bb
    for nm, off in (("A1", L1W_OFF), ("A2", L2W_OFF)):
        A = W2[:, off:off + 16384].reshape(LORA, 128, 128)
        d[nm] = bf(np.transpose(A, (1, 0, 2)).reshape(128, LORA * 128))
    return d


# The branch weights / biases / W2-derived tensors are identical across
# cores; prep them once and share the arrays between in_maps.
def _make_in_maps(inputs):
    shared = None
    maps = []
    for core in range(N_CORES):
        d = _host_prep(inputs, core)
        if shared is None:
            shared = {k: d[k] for k in d
                      if k not in ("uF", "u0", "tb")}
        else:
            for k in shared:
                d[k] = shared[k]
        maps.append(d)
    return maps


def kernel(**inputs):
    from concourse.bass_utils import run_bass_kernel_spmd

    inputs = {k: np.asarray(v) for k, v in inputs.items()}
    nc = _get_program()
    in_maps = _make_in_maps(inputs)
    res = None
    last_err = None
    for attempt in range(3):
        try:
            res = run_bass_kernel_spmd(nc, in_maps, core_ids=list(range(N_CORES)))
            break
        except Exception as e:  # transient NRT/device hiccups recover on retry
            last_err = e
    if res is None:
        raise last_err
    outs = []
    for core in range(N_CORES):
        oc = np.asarray(res.results[core]["out"], np.float32)  # [128, 2]
        outs.append(oc.T.reshape(BL))
    return np.concatenate(outs).astype(np.float32)
